# revision 33
# baseline (speedup 1.0000x reference)
"""Trainium2 Bass kernel for nn_LiveNet: 2-layer MLP
    y = relu(relu(x @ W1.T + b1) @ W2.T + b2)
  x [65536, 512], W1 [128, 512], b1 [128], W2 [64, 128], b2 [64] -> y [65536, 64]

Strategy (data-parallel over 8 cores, 8192 rows each):
  Work in the transposed space so the contraction dims land on SBUF
  partitions with no on-chip transposes at all:
      hT = relu(W1T.T @ xT + b1)     [128, n]
      yT = relu(W2T.T @ hT + b2)     [64, n]
  Host pre-tiles x per core into [NCH, 128, KC, NB] (chunk-contiguous
  partition rows -> large DMA descriptors) and converts x/W to float16
  (halves HBM traffic; PE streams fp16 at 1 cyc/row vs 2 for fp32; rel
  err ~4e-4 since products accumulate in fp32 PSUM); kernel writes
  yT [64, 8192]; host transposes back to the fp32 [B, 64] output.
"""

import numpy as np

N_CORES = 8
B, D_IN, D_MID, D_OUT = 65536, 512, 128, 64
BS = B // N_CORES        # rows per core
NB = 512                 # batch-columns per inner tile
KC = D_IN // 128         # K-chunks for layer 1
NCH = BS // NB

_compiled = None


def _build_nc():
    import concourse.bass as bass
    import concourse.mybir as mybir
    import concourse.tile as tile
    from concourse import bacc

    f32 = mybir.dt.float32
    f32r = mybir.dt.float16  # fp16: 1 cyc/row, half DMA traffic, 10-bit mantissa
    relu = mybir.ActivationFunctionType.Relu

    # Trim the Tile exit sequence: skip the semaphore re-clear and second
    # all-engine barrier (the per-launch Bass preamble clears semaphores
    # before every execution, so the exit clear is redundant for a
    # standalone NEFF). Keeps drain + one barrier so all engines/DMAs are
    # quiesced before the NEFF ends.
    def _fast_drain(self, tick_clock, wait_clock):
        drain_inst = self.nc.sync.drain()
        wait_clock.add_sem_waits(
            drain_inst.ins, tile.ScopedClock({None: tick_clock.global_clock})
        )
        self.nc.all_engine_barrier()
        popped = self.nc._tile_sem_poison_stack.pop()
        assert popped is self._sem_poison

    tile.TileContext._drain_and_barrier = _fast_drain

    nc = bacc.Bacc(
        "TRN2", target_bir_lowering=False, debug=False, num_devices=N_CORES
    )
    xT = nc.dram_tensor("xT", [NCH, 128, KC, NB], f32r, kind="ExternalInput").ap()
    w1T = nc.dram_tensor("w1T", [D_IN, D_MID], f32r, kind="ExternalInput").ap()
    b1 = nc.dram_tensor("b1", [D_MID, 1], f32, kind="ExternalInput").ap()
    w2T = nc.dram_tensor("w2T", [D_MID, D_OUT], f32r, kind="ExternalInput").ap()
    b2 = nc.dram_tensor("b2", [D_OUT, 1], f32, kind="ExternalInput").ap()
    yT = nc.dram_tensor("yT", [D_OUT, BS], f32, kind="ExternalOutput").ap()

    with tile.TileContext(nc) as tc:
        with (
            tc.tile_pool(name="const", bufs=1) as cpool,
            tc.tile_pool(name="xin", bufs=8) as xpool,
            tc.tile_pool(name="hid", bufs=6) as hpool,
            tc.tile_pool(name="yout", bufs=4) as ypool,
            tc.tile_pool(name="ph", bufs=4, space=bass.MemorySpace.PSUM) as phpool,
            tc.tile_pool(name="py", bufs=2, space=bass.MemorySpace.PSUM) as pypool,
        ):
            # Constants first on the same HWDGE queue as the x loads (FIFO
            # -> they complete before the x flood).
            w1s = cpool.tile([128, KC, D_MID], f32r)
            nc.sync.dma_start(w1s[:], w1T.rearrange("(k p) m -> p k m", p=128))
            w2s = cpool.tile([D_MID, D_OUT], f32r)
            nc.sync.dma_start(w2s[:], w2T)
            b1s = cpool.tile([D_MID, 1], f32)
            nc.sync.dma_start(b1s[:], b1)
            b2s = cpool.tile([D_OUT, 1], f32)
            nc.sync.dma_start(b2s[:], b2)

            # Group chunks so each weight tile is loaded once per group and
            # streams G chunks back-to-back (k-outer): 5 LDWEIGHTS per group
            # instead of 5 per chunk.
            G = 4
            for g in range(NCH // G):
                xts = []
                for i in range(G):
                    xt = xpool.tile([128, KC, NB], f32r, name=f"xt{g}_{i}", tag="xt")
                    nc.sync.dma_start(xt[:], xT[g * G + i])
                    xts.append(xt)

                phs = [phpool.tile([D_MID, NB], f32, name=f"ph{g}_{i}", tag="ph") for i in range(G)]
                for k in range(KC):
                    for i in range(G):
                        nc.tensor.matmul(
                            phs[i][:], w1s[:, k, :], xts[i][:, k, :],
                            start=(k == 0), stop=(k == KC - 1),
                        )
                hts = []
                for i in range(G):
                    ht = hpool.tile([D_MID, NB], f32r, name=f"ht{g}_{i}", tag="ht")
                    nc.scalar.activation(ht[:], phs[i][:], relu, bias=b1s[:])
                    hts.append(ht)

                for i in range(G):
                    ib = g * G + i
                    py = pypool.tile([D_OUT, NB], f32, name=f"py{g}_{i}", tag="py")
                    nc.tensor.matmul(py[:], w2s[:], hts[i][:], start=True, stop=True)
                    yt = ypool.tile([D_OUT, NB], f32, name=f"yt{g}_{i}", tag="yt")
                    nc.vector.tensor_scalar(
                        yt[:], py[:], b2s[:D_OUT], 0.0,
                        mybir.AluOpType.add, op1=mybir.AluOpType.max,
                    )
                    nc.gpsimd.dma_start(yT[:, ib * NB:(ib + 1) * NB], yt[:])

    nc.compile()
    return nc


def _get_nc():
    global _compiled
    if _compiled is None:
        _compiled = _build_nc()
    return _compiled


def _run(x, W1, b1, W2, b2, **kw):
    from concourse.bass_utils import run_bass_kernel_spmd

    x = np.ascontiguousarray(np.asarray(x, dtype=np.float32).astype(np.float16))
    w1T = np.ascontiguousarray(np.asarray(W1, dtype=np.float32).T.astype(np.float16))
    w2T = np.ascontiguousarray(np.asarray(W2, dtype=np.float32).T.astype(np.float16))
    b1c = np.ascontiguousarray(np.asarray(b1, dtype=np.float32).reshape(D_MID, 1))
    b2c = np.ascontiguousarray(np.asarray(b2, dtype=np.float32).reshape(D_OUT, 1))

    in_maps = []
    for c in range(N_CORES):
        # [NCH, 128, KC, NB]: per-chunk fully contiguous partition rows so
        # each chunk load is 128 descriptors of KC*NB*4 bytes.
        Xc = x[c * BS:(c + 1) * BS]
        xTc = np.ascontiguousarray(
            Xc.reshape(NCH, NB, KC, 128).transpose(0, 3, 2, 1)
        )
        in_maps.append({"xT": xTc, "w1T": w1T, "b1": b1c, "w2T": w2T, "b2": b2c})

    res = run_bass_kernel_spmd(_get_nc(), in_maps, list(range(N_CORES)), **kw)

    y = np.empty((B, D_OUT), dtype=np.float32)
    for c in range(N_CORES):
        y[c * BS:(c + 1) * BS] = res.results[c]["yT"].T
    return y, res


def kernel(x, W1, b1, W2, b2):
    y, _ = _run(x, W1, b1, W2, b2)
    return y


# revision 34
# speedup vs baseline: 1.1290x; 1.1290x over previous
"""Trainium2 Bass kernel for nn_LiveNet: 2-layer MLP
    y = relu(relu(x @ W1.T + b1) @ W2.T + b2)
  x [65536, 512], W1 [128, 512], b1 [128], W2 [64, 128], b2 [64] -> y [65536, 64]

Strategy (data-parallel over 8 cores, 8192 rows each):
  Work in the transposed space so the contraction dims land on SBUF
  partitions with no on-chip transposes at all:
      hT = relu(W1T.T @ xT + b1)     [128, n]
      yT = relu(W2T.T @ hT + b2)     [64, n]
  Host pre-tiles x per core into [NCH, 128, KC, NB] (chunk-contiguous
  partition rows -> large DMA descriptors) and converts x/W to float16
  (halves HBM traffic; PE streams fp16 at 1 cyc/row vs 2 for fp32; rel
  err ~4e-4 since products accumulate in fp32 PSUM); kernel writes
  yT [64, 8192]; host transposes back to the fp32 [B, 64] output.
"""

import numpy as np

N_CORES = 8
B, D_IN, D_MID, D_OUT = 65536, 512, 128, 64
BS = B // N_CORES        # rows per core
NB = 512                 # batch-columns per inner tile
KC = D_IN // 128         # K-chunks for layer 1
NCH = BS // NB

_compiled = None


def _build_nc():
    import concourse.bass as bass
    import concourse.mybir as mybir
    import concourse.tile as tile
    from concourse import bacc

    f32 = mybir.dt.float32
    f32r = mybir.dt.float16  # fp16: 1 cyc/row, half DMA traffic, 10-bit mantissa
    relu = mybir.ActivationFunctionType.Relu

    nc = bacc.Bacc(
        "TRN2", target_bir_lowering=False, debug=False, num_devices=N_CORES
    )
    xT = nc.dram_tensor("xT", [NCH, 128, KC, NB], f32r, kind="ExternalInput").ap()
    w1T = nc.dram_tensor("w1T", [D_IN, D_MID], f32r, kind="ExternalInput").ap()
    b1 = nc.dram_tensor("b1", [D_MID, 1], f32, kind="ExternalInput").ap()
    w2T = nc.dram_tensor("w2T", [D_MID, D_OUT], f32r, kind="ExternalInput").ap()
    b2 = nc.dram_tensor("b2", [D_OUT, 1], f32, kind="ExternalInput").ap()
    yT = nc.dram_tensor("yT", [D_OUT, BS], f32, kind="ExternalOutput").ap()

    with tile.TileContext(nc) as tc:
        with (
            tc.tile_pool(name="const", bufs=1) as cpool,
            tc.tile_pool(name="xin", bufs=8) as xpool,
            tc.tile_pool(name="hid", bufs=6) as hpool,
            tc.tile_pool(name="yout", bufs=4) as ypool,
            tc.tile_pool(name="ph", bufs=4, space=bass.MemorySpace.PSUM) as phpool,
            tc.tile_pool(name="py", bufs=2, space=bass.MemorySpace.PSUM) as pypool,
        ):
            # Constants first on the same HWDGE queue as the x loads (FIFO
            # -> they complete before the x flood).
            w1s = cpool.tile([128, KC, D_MID], f32r)
            nc.sync.dma_start(w1s[:], w1T.rearrange("(k p) m -> p k m", p=128))
            w2s = cpool.tile([D_MID, D_OUT], f32r)
            nc.sync.dma_start(w2s[:], w2T)
            b1s = cpool.tile([D_MID, 1], f32)
            nc.sync.dma_start(b1s[:], b1)
            b2s = cpool.tile([D_OUT, 1], f32)
            nc.sync.dma_start(b2s[:], b2)

            # Group chunks so each weight tile is loaded once per group and
            # streams G chunks back-to-back (k-outer): 5 LDWEIGHTS per group
            # instead of 5 per chunk.
            G = 4
            for g in range(NCH // G):
                xts = []
                for i in range(G):
                    xt = xpool.tile([128, KC, NB], f32r, name=f"xt{g}_{i}", tag="xt")
                    nc.sync.dma_start(xt[:], xT[g * G + i])
                    xts.append(xt)

                phs = [phpool.tile([D_MID, NB], f32, name=f"ph{g}_{i}", tag="ph") for i in range(G)]
                for k in range(KC):
                    for i in range(G):
                        nc.tensor.matmul(
                            phs[i][:], w1s[:, k, :], xts[i][:, k, :],
                            start=(k == 0), stop=(k == KC - 1),
                        )
                hts = []
                for i in range(G):
                    ht = hpool.tile([D_MID, NB], f32r, name=f"ht{g}_{i}", tag="ht")
                    nc.scalar.activation(ht[:], phs[i][:], relu, bias=b1s[:])
                    hts.append(ht)

                for i in range(G):
                    ib = g * G + i
                    py = pypool.tile([D_OUT, NB], f32, name=f"py{g}_{i}", tag="py")
                    nc.tensor.matmul(py[:], w2s[:], hts[i][:], start=True, stop=True)
                    yt = ypool.tile([D_OUT, NB], f32, name=f"yt{g}_{i}", tag="yt")
                    nc.vector.tensor_scalar(
                        yt[:], py[:], b2s[:D_OUT], 0.0,
                        mybir.AluOpType.add, op1=mybir.AluOpType.max,
                    )
                    nc.gpsimd.dma_start(yT[:, ib * NB:(ib + 1) * NB], yt[:])

    nc.compile()
    return nc


def _get_nc():
    global _compiled
    if _compiled is None:
        _compiled = _build_nc()
    return _compiled


def _run(x, W1, b1, W2, b2, **kw):
    from concourse.bass_utils import run_bass_kernel_spmd

    x = np.ascontiguousarray(np.asarray(x, dtype=np.float32).astype(np.float16))
    w1T = np.ascontiguousarray(np.asarray(W1, dtype=np.float32).T.astype(np.float16))
    w2T = np.ascontiguousarray(np.asarray(W2, dtype=np.float32).T.astype(np.float16))
    b1c = np.ascontiguousarray(np.asarray(b1, dtype=np.float32).reshape(D_MID, 1))
    b2c = np.ascontiguousarray(np.asarray(b2, dtype=np.float32).reshape(D_OUT, 1))

    in_maps = []
    for c in range(N_CORES):
        # [NCH, 128, KC, NB]: per-chunk fully contiguous partition rows so
        # each chunk load is 128 descriptors of KC*NB*4 bytes.
        Xc = x[c * BS:(c + 1) * BS]
        xTc = np.ascontiguousarray(
            Xc.reshape(NCH, NB, KC, 128).transpose(0, 3, 2, 1)
        )
        in_maps.append({"xT": xTc, "w1T": w1T, "b1": b1c, "w2T": w2T, "b2": b2c})

    res = run_bass_kernel_spmd(_get_nc(), in_maps, list(range(N_CORES)), **kw)

    y = np.empty((B, D_OUT), dtype=np.float32)
    for c in range(N_CORES):
        y[c * BS:(c + 1) * BS] = res.results[c]["yT"].T
    return y, res


def kernel(x, W1, b1, W2, b2):
    y, _ = _run(x, W1, b1, W2, b2)
    return y
